# revision 23
# baseline (speedup 1.0000x reference)
"""Trainium2 Bass kernel for the HAM module.

Mathematical collapse used here (exact algebra, not an approximation):
  attn_w = softmax(attn, axis=K)  =>  sum_k attn_w[b,k,c] == 1 for every (b,c)
  attn_out_avg[b,s] = mean_k sum_c attn_w[b,k,c] x[b,c,s]
                    = sum_c (1/K) x[b,c,s] = (C/K) * x_mean[b,s] = 4 * x_mean
so local_input = [x_mean, 4*x_mean] and none of the three outputs depend on
t / attn / omega_h / W_s1 / W_s2 at all.  The whole network reduces to:
  x_sum[b,s] = sum_c x[b,c,s]                       (the only touch of x, 96MB)
  fc    = x_sum @ W_effT + b_t,  W_eff = (W_t[:,:S] + 4*W_t[:,S:]) / C
  BN over the full batch (global stats), relu      -> local_fc_out
  sigmoid(lfc @ W_l.T + b_l)                       -> local_score
  (local_score @ G_h) broadcast along A            -> omega_h_next

Distribution: batch-parallel over 8 cores (8 samples each) for the x
reduction; one small AllGather of the per-core [8,768] x_sum slices; the tiny
tail (fc/BN/score/Q) is computed replicated on every core; each core writes
only its own batch slice of omega_h_next (selected with a per-core one-hot
matmul, no dynamic addressing).
"""

import numpy as np

import concourse.bacc as bacc
import concourse.bass as bass
import concourse.bass_utils as bass_utils
import concourse.mybir as mybir
import concourse.tile as tile

F32 = mybir.dt.float32
AFT = mybir.ActivationFunctionType
AXX = mybir.AxisListType

N_CORES = 8
B, C, S = 64, 512, 768
H, K, KN, A = 1024, 128, 256, 200
BL = B // N_CORES          # 8 samples per core
NSC = S // 128             # 6 s-chunks
NHC = H // 128             # 8 h-chunks
NCC = C // 128             # 4 c-chunks
BN_EPS = 1e-5

_CACHED_NC = None


def _build_nc(for_sim=False):
    nc = bacc.Bacc(
        "TRN2",
        debug=False,
        enable_asserts=True,
        target_bir_lowering=False,
        num_devices=N_CORES,
    )

    xs = nc.dram_tensor("xs", [BL, C, S], F32, kind="ExternalInput")
    er = nc.dram_tensor("er", [B, BL], F32, kind="ExternalInput")
    wefft = nc.dram_tensor("wefft", [S, H], F32, kind="ExternalInput")
    wlt = nc.dram_tensor("wlt", [H, K], F32, kind="ExternalInput")
    gh = nc.dram_tensor("gh", [K, KN], F32, kind="ExternalInput")
    btv = nc.dram_tensor("btv", [128, NHC], F32, kind="ExternalInput")
    gammav = nc.dram_tensor("gammav", [128, NHC], F32, kind="ExternalInput")
    betav = nc.dram_tensor("betav", [128, NHC], F32, kind="ExternalInput")
    blv = nc.dram_tensor("blv", [128, 1], F32, kind="ExternalInput")
    ident = nc.dram_tensor("ident", [128, 128], F32, kind="ExternalInput")

    score_out = nc.dram_tensor("score_out", [B, K], F32, kind="ExternalOutput")
    lfc_out = nc.dram_tensor("lfc_out", [B, H], F32, kind="ExternalOutput")
    omega_out = nc.dram_tensor("omega_out", [BL, KN, A], F32, kind="ExternalOutput")

    with tile.TileContext(nc) as tc:
        with (
            tc.tile_pool(name="cpool", bufs=1) as cpool,
            tc.tile_pool(name="xpool", bufs=8) as xpool,
            tc.tile_pool(name="spool", bufs=2) as spool,
            tc.tile_pool(name="pp", bufs=2, space="PSUM") as pp,
            tc.tile_pool(name="ppacc", bufs=1, space="PSUM") as ppacc,
            tc.tile_pool(name="dpool", bufs=1, space="DRAM") as dpool,
        ):
            # ---- constants / weights ----
            ones = cpool.tile([128, A], F32, tag="ones")
            nc.vector.memset(ones[:, :], 1.0)
            epsv = cpool.tile([128, 1], F32, tag="epsv")
            nc.vector.memset(epsv[:, :], BN_EPS)
            ident_sb = cpool.tile([128, 128], F32, tag="ident")
            nc.sync.dma_start(ident_sb[:, :], ident.ap())
            wefft_sb = cpool.tile([128, NSC, H], F32, tag="wefft")
            nc.sync.dma_start(
                wefft_sb[:, :, :], wefft.ap().rearrange("(sc p) h -> p sc h", p=128)
            )
            wlt_sb = cpool.tile([128, NHC, K], F32, tag="wlt")
            nc.sync.dma_start(
                wlt_sb[:, :, :], wlt.ap().rearrange("(hc p) k -> p hc k", p=128)
            )
            gh_sb = cpool.tile([128, KN], F32, tag="gh")
            nc.sync.dma_start(gh_sb[:, :], gh.ap())
            btv_sb = cpool.tile([128, NHC], F32, tag="btv")
            nc.sync.dma_start(btv_sb[:, :], btv.ap())
            gammav_sb = cpool.tile([128, NHC], F32, tag="gammav")
            nc.sync.dma_start(gammav_sb[:, :], gammav.ap())
            betav_sb = cpool.tile([128, NHC], F32, tag="betav")
            nc.sync.dma_start(betav_sb[:, :], betav.ap())
            blv_sb = cpool.tile([128, 1], F32, tag="blv")
            nc.sync.dma_start(blv_sb[:, :], blv.ap())
            er_sb = cpool.tile([B, BL], F32, tag="er")
            nc.sync.dma_start(er_sb[:, :], er.ap())

            # ---- phase 1: x_sum[b, s] = sum_c x[b, c, s] via ones-matmul ----
            ag_in = dpool.tile([BL, S], F32, tag="ag_in")
            ag_out = dpool.tile([B, S], F32, tag="ag_out")
            for b in range(BL):
                xt = xpool.tile([128, NCC, S], F32, tag="xt")
                nc.sync.dma_start(
                    xt[:, :, :], xs.ap()[b].rearrange("(cc p) s -> p cc s", p=128)
                )
                xsum_row = spool.tile([1, S], F32, tag="xsum_row", bufs=3)
                # partial reduction over the 4 c-chunks on DVE, then one
                # 128-way partition reduction per half on PE
                xr = spool.tile([128, S], F32, tag="xr", bufs=3)
                nc.vector.tensor_add(xr[:, :], xt[:, 0, :], xt[:, 1, :])
                nc.vector.tensor_add(xr[:, :], xr[:, :], xt[:, 2, :])
                nc.vector.tensor_add(xr[:, :], xr[:, :], xt[:, 3, :])
                for hf in range(2):
                    pxs = pp.tile([1, 384], F32, tag="pxs", bufs=2)
                    nc.tensor.matmul(
                        pxs[:, :],
                        lhsT=ones[:, 0:1],
                        rhs=xr[:, hf * 384 : (hf + 1) * 384],
                        start=True,
                        stop=True,
                    )
                    if hf == 0:
                        nc.scalar.copy(
                            xsum_row[:, hf * 384 : (hf + 1) * 384], pxs[:, :]
                        )
                    else:
                        nc.vector.tensor_copy(
                            xsum_row[:, hf * 384 : (hf + 1) * 384], pxs[:, :]
                        )
                nc.sync.dma_start(ag_in[b : b + 1, :], xsum_row[:, :])

            # ---- phase 2: AllGather of x_sum slices ----
            if for_sim:
                # TimelineSim can't model collectives; stand in a DMA to keep deps
                nc.sync.dma_start(ag_out[0:BL, :], ag_in[:, :])
            else:
                nc.gpsimd.collective_compute(
                    "AllGather",
                    mybir.AluOpType.bypass,
                    replica_groups=[list(range(N_CORES))],
                    ins=[ag_in.opt()],
                    outs=[ag_out.opt()],
                )
            xm_sb = spool.tile([B, S], F32, tag="xm")
            nc.sync.dma_start(xm_sb[:, :], ag_out[:, :])

            # ---- phase 3: transpose x_sum -> xsT [128, sc*B] ----
            xsT = spool.tile([128, NSC, B], F32, tag="xsT")
            for sc in range(NSC):
                pt = pp.tile([128, B], F32, tag="tp")
                nc.tensor.transpose(
                    pt[:, :], xm_sb[:, sc * 128 : (sc + 1) * 128], ident_sb[:B, :B]
                )
                nc.scalar.copy(xsT[:, sc, :], pt[:, :])

            # ---- phase 4: fc.T chunks (psum drain + b_t bias into one 3D tile) ----
            lfc_sb = spool.tile([B, H], F32, tag="lfc")
            psc = ppacc.tile([K, B], F32, tag="psc")
            fcT_all = spool.tile([128, NHC, B], F32, tag="fcT_all")
            for hc in range(NHC):
                pfc = pp.tile([128, B], F32, tag="pfc", bufs=3)
                for sc in range(NSC):
                    nc.tensor.matmul(
                        pfc[:, :],
                        lhsT=wefft_sb[:, sc, hc * 128 : (hc + 1) * 128],
                        rhs=xsT[:, sc, :],
                        start=(sc == 0),
                        stop=(sc == NSC - 1),
                    )
                nc.scalar.activation(
                    fcT_all[:, hc, :], pfc[:, :], AFT.Identity,
                    bias=btv_sb[:, hc : hc + 1],
                )
            # ---- phase 5: batchnorm over the full batch, all chunks at once ----
            mu_r = spool.tile([128, NHC], F32, tag="mu_r")
            nc.vector.reduce_sum(mu_r[:, :], fcT_all[:, :, :], axis=AXX.X)
            mu = spool.tile([128, NHC], F32, tag="mu")
            nc.scalar.mul(mu[:, :], mu_r[:, :], 1.0 / B)
            sq_all = spool.tile([128, NHC, B], F32, tag="sq_all")
            nc.vector.tensor_mul(sq_all[:, :, :], fcT_all[:, :, :], fcT_all[:, :, :])
            ss_r = spool.tile([128, NHC], F32, tag="ss_r")
            nc.vector.reduce_sum(ss_r[:, :], sq_all[:, :, :], axis=AXX.X)
            musq = spool.tile([128, NHC], F32, tag="musq")
            nc.vector.tensor_mul(musq[:, :], mu[:, :], mu[:, :])
            va = spool.tile([128, NHC], F32, tag="va")
            nc.vector.tensor_scalar(
                va[:, :], ss_r[:, :], 1.0 / B, None, op0=mybir.AluOpType.mult
            )
            nc.vector.tensor_sub(va[:, :], va[:, :], musq[:, :])
            stdv = spool.tile([128, NHC], F32, tag="stdv")
            nc.scalar.activation(stdv[:, :], va[:, :], AFT.Sqrt, bias=epsv[:, :])
            rstd = spool.tile([128, NHC], F32, tag="rstd")
            nc.vector.reciprocal(rstd[:, :], stdv[:, :])
            sh = spool.tile([128, NHC, 1], F32, tag="sh")
            nc.vector.tensor_mul(sh[:, :, 0], rstd[:, :], gammav_sb[:, :])
            t2 = spool.tile([128, NHC], F32, tag="t2")
            nc.vector.tensor_mul(t2[:, :], mu[:, :], sh[:, :, 0])
            bias2 = spool.tile([128, NHC, 1], F32, tag="bias2")
            nc.vector.tensor_sub(bias2[:, :, 0], betav_sb[:, :], t2[:, :])
            # lfcT = relu(fcT*sh + bias2), broadcast [128,8,1] -> [128,8,64]
            bn_all = spool.tile([128, NHC, B], F32, tag="bn_all")
            nc.vector.tensor_mul(
                bn_all[:, :, :], fcT_all[:, :, :],
                sh[:, :, :].to_broadcast((128, NHC, B)),
            )
            nc.vector.tensor_add(
                bn_all[:, :, :], bn_all[:, :, :],
                bias2[:, :, :].to_broadcast((128, NHC, B)),
            )
            lfcT_all = spool.tile([128, NHC, B], F32, tag="lfcT_all")
            nc.scalar.activation(lfcT_all[:, :, :], bn_all[:, :, :], AFT.Relu)

            # ---- phase 6: score partials + lfc output transposes ----
            for hc in range(NHC):
                nc.tensor.matmul(
                    psc[:, :],
                    lhsT=wlt_sb[:, hc, :],
                    rhs=lfcT_all[:, hc, :],
                    start=(hc == 0),
                    stop=(hc == NHC - 1),
                )
                plf = pp.tile([B, 128], F32, tag="tp")
                nc.tensor.transpose(plf[:, :], lfcT_all[:, hc, :], ident_sb[:, :])
                nc.scalar.copy(lfc_sb[:, hc * 128 : (hc + 1) * 128], plf[:, :])
            nc.sync.dma_start(lfc_out.ap(), lfc_sb[:, :])

            # ---- phase 7: score, own-batch selection, Q, omega ----
            scT = spool.tile([K, B], F32, tag="scT")
            nc.scalar.activation(
                scT[:, :], psc[:, :], AFT.Sigmoid, bias=blv_sb[:, 0:1]
            )
            pt2 = pp.tile([B, K], F32, tag="tp")
            nc.tensor.transpose(pt2[:, :], scT[:, :], ident_sb[:, :])
            score_sb = spool.tile([B, K], F32, tag="score")
            nc.scalar.copy(score_sb[:, :], pt2[:, :])
            nc.sync.dma_start(score_out.ap(), score_sb[:, :])

            # own columns: scoreT_own[k, i] = sum_b score[b, k] er[b, i]
            pown = pp.tile([K, BL], F32, tag="tp")
            nc.tensor.matmul(
                pown[:, :], lhsT=score_sb[:, :], rhs=er_sb[:, :], start=True, stop=True
            )
            sown = spool.tile([K, BL], F32, tag="sown")
            nc.scalar.copy(sown[:, :], pown[:, :])

            # QownT[k', i] = sum_k G_h[k, k'] scoreT_own[k, i]
            qT = spool.tile([128, 2 * BL], F32, tag="qT")
            for half in range(2):
                pq = pp.tile([128, BL], F32, tag="tp")
                nc.tensor.matmul(
                    pq[:, :],
                    lhsT=gh_sb[:, half * 128 : (half + 1) * 128],
                    rhs=sown[:, :],
                    start=True,
                    stop=True,
                )
                nc.scalar.copy(qT[:, half * BL : (half + 1) * BL], pq[:, :])

            # omega_out[i, k', a] = Qown[i, k']  (broadcast along A)
            # one DVE op builds all 16 tiles; one DMA writes all 1.64MB
            om_all = spool.tile([128, 2 * BL, A], F32, tag="om_all")
            nc.vector.tensor_scalar_mul(
                om_all[:, :, :],
                qT[:, :].to_broadcast((128, 2 * BL, A)),
                1.0,
            )
            for half in range(2):
                nc.sync.dma_start(
                    omega_out.ap()[:, half * 128 : (half + 1) * 128, :].rearrange(
                        "bo p a -> p bo a"
                    ),
                    om_all[:, half * BL : (half + 1) * BL, :],
                )

    nc.compile()
    return nc


def _get_nc():
    global _CACHED_NC
    if _CACHED_NC is None:
        _CACHED_NC = _build_nc()
    return _CACHED_NC


def _prep_in_maps(inputs):
    x = np.asarray(inputs["x"], dtype=np.float32)
    W_t = np.asarray(inputs["W_t"], dtype=np.float32)
    b_t = np.asarray(inputs["b_t"], dtype=np.float32)
    gamma = np.asarray(inputs["gamma"], dtype=np.float32)
    beta = np.asarray(inputs["beta"], dtype=np.float32)
    W_l = np.asarray(inputs["W_l"], dtype=np.float32)
    b_l = np.asarray(inputs["b_l"], dtype=np.float32)
    G_h = np.asarray(inputs["G_h"], dtype=np.float32)

    W_eff = (W_t[:, :S] + 4.0 * W_t[:, S:]) / float(C)   # [H, S]
    wefft = np.ascontiguousarray(W_eff.T)                # [S, H]
    wlt = np.ascontiguousarray(W_l.T)                    # [H, K]
    gh = np.ascontiguousarray(G_h)                       # [K, KN]
    btv = np.ascontiguousarray(b_t.reshape(NHC, 128).T)
    gammav = np.ascontiguousarray(gamma.reshape(NHC, 128).T)
    betav = np.ascontiguousarray(beta.reshape(NHC, 128).T)
    blv = np.ascontiguousarray(b_l.reshape(128, 1))
    ident = np.eye(128, dtype=np.float32)

    in_maps = []
    for r in range(N_CORES):
        er = np.zeros((B, BL), np.float32)
        er[BL * r + np.arange(BL), np.arange(BL)] = 1.0
        in_maps.append(
            {
                "xs": np.ascontiguousarray(x[BL * r : BL * (r + 1)]),
                "er": er,
                "wefft": wefft,
                "wlt": wlt,
                "gh": gh,
                "btv": btv,
                "gammav": gammav,
                "betav": betav,
                "blv": blv,
                "ident": ident,
            }
        )
    return in_maps


def run(inputs, **spmd_kwargs):
    """Compile (cached), run on 8 cores, return (outputs, BassKernelResults)."""
    nc = _get_nc()
    in_maps = _prep_in_maps(inputs)
    res = bass_utils.run_bass_kernel_spmd(
        nc, in_maps, core_ids=list(range(N_CORES)), **spmd_kwargs
    )
    score = np.asarray(res.results[0]["score_out"])
    lfc = np.asarray(res.results[0]["lfc_out"])
    omega = np.concatenate(
        [np.asarray(res.results[r]["omega_out"]) for r in range(N_CORES)], axis=0
    )
    return (score, lfc, omega), res


def kernel(**inputs):
    outs, _ = run(inputs)
    return outs
